# revision 14
# baseline (speedup 1.0000x reference)
"""MMoE layer kernel for 8 Trainium2 NeuronCores.

Reference math (B=4096, D=1024, H1=2048, H2=1024, E=7 experts, NS=7 scenes):
  h        = relu(einsum('bd,edh', x, W1) + b1)           # [B,E,H1]
  eo       = relu(einsum('beh,eho', h, W2) + b2)          # [B,E,H2]
  xc       = concat(x, scene_emb[scene])                  # [B, D+16]
  G        = softmax over s of einsum('bd,sde', xc, S)    # [B,E,NS] (after transpose)
  q        = mean_s log(G*7)                              # [B,E]
  score1   = logG[b, e, scene_b]
  select   = drop expert e iff e == argmin_e score1 == argmin_e q
  gate     = softmax_e(exp(score1)) * select
  out      = einsum('be,beo', gate, eo); output = stack([out, out])

Sharding: data-parallel over batch (512 rows/core), weights replicated.

Precision: expert matmuls run in bf16 except the first FP8_K=256 rows
of BOTH contractions (layer 1: x/W1, layer 2: h/W2), which run as one
fp8e4 DoubleRow matmul each (2 k-tiles per instruction, ~1.9x measured).
Unscaled e4m3 keeps the fp8 partial products in the same PSUM scale as
the bf16 ones, so they share one accumulation group. Measured end-to-end
metric 1.909e-2 vs the 2e-2 gate (deterministic; bit-stable across runs
and exactly predicted by the numpy simulation of the same quantization).

Routing runs reversed ([49, B] out = S^T x) as bf16 hi/lo 3-term matmuls
(Sh*xh + Sl*xh + Sh*xl, max logit err ~1.3e-5, 40-100x below the
smallest argmin gap so select stays bit-stable), then PE-transposes back
to [128, 49]-per-b-tile for the fp32 gate chain. Terms 1-2 run before
L1(e0) (their inputs lead the DMA queue), the xl term + scene-table term
after L1(e0): together with ~24 warmup matmuls this keeps the PE busy
through the DMA-bound prologue and holds the HAM clock gate at 8/8.
All inputs are repacked partition-major on the host so every prologue
DMA moves few large descriptors (the naive [d, b] layouts made the
prologue descriptor-bound and ~2x slower).
"""

import sys

if "/opt/trn_rl_repo" not in sys.path:
    sys.path.insert(0, "/opt/trn_rl_repo")

from contextlib import ExitStack

import ml_dtypes
import numpy as np

import concourse.bass as bass
import concourse.tile as tile
from concourse import bacc, mybir
from concourse.bass_utils import run_bass_kernel_spmd

F32 = mybir.dt.float32
BF16 = mybir.dt.bfloat16
FP8 = mybir.dt.float8e4
AF = mybir.ActivationFunctionType
ALU = mybir.AluOpType
AX = mybir.AxisListType
DR = mybir.MatmulPerfMode.DoubleRow

N_CORES = 8
B, D, H1, H2, E, NS, T = 4096, 1024, 2048, 1024, 7, 7, 2
BL = B // N_CORES          # 512 rows per core
NB = BL // 128             # 4 batch tiles
FP8_K = 256                # leading K rows of layer 1 in fp8 (one DoubleRow pair)
KT8 = FP8_K // 128         # 2 fp8 k-tiles
KTB = (D - FP8_K) // 128   # 6 bf16 k-tiles, layer 1
KT = D // 128              # 8 k-tiles of x (routing)
MT1 = H1 // 128            # 16 m-tiles, layer 1
NQ = 4                     # w1 column quarters for expert 0 (DMA granularity)
QW = H1 // NQ              # 512 columns per quarter
KT2 = H1 // 128            # 16 k-tiles, layer 2
NO = H2 // 512             # 2  512-wide out column blocks
EN = E * NS                # 49
WARM = 24                  # HAM clock-ramp matmuls while the DMA queue boots
NP_BF16 = np.dtype(ml_dtypes.bfloat16)
NP_FP8 = np.dtype(ml_dtypes.float8_e4m3)


def _emit_kernel(tc, aps, has_b1, has_b2):
    nc = tc.nc
    ctx = ExitStack()
    with ctx:
        consts = ctx.enter_context(tc.tile_pool(name="consts", bufs=1))
        w18pool = ctx.enter_context(tc.tile_pool(name="w18", bufs=2))
        w1pool = ctx.enter_context(tc.tile_pool(name="w1", bufs=2))
        w2pool = ctx.enter_context(tc.tile_pool(name="w2", bufs=1))
        htpool = ctx.enter_context(tc.tile_pool(name="ht", bufs=1))
        tmppool = ctx.enter_context(tc.tile_pool(name="tmp", bufs=3))
        l1ps = ctx.enter_context(tc.tile_pool(name="l1ps", bufs=4, space="PSUM"))
        l2ps = ctx.enter_context(tc.tile_pool(name="l2ps", bufs=4, space="PSUM"))
        rpool = tc.alloc_tile_pool(name="routing", bufs=1)

        # ---- DMA queue in consumption order: routing inputs, then expert-0
        # L1 inputs, then everything else. -------------------------------
        sfh_sb = rpool.tile([128, KT, EN], BF16)
        nc.sync.dma_start(sfh_sb[:, :, :], aps["sfh"][:, :])
        sfl_sb = rpool.tile([128, KT, EN], BF16)
        nc.sync.dma_start(sfl_sb[:, :, :], aps["sfl"][:, :])
        xfull_sb = consts.tile([128, KT, BL], BF16)
        nc.sync.dma_start(xfull_sb[:, :, :], aps["xh"][:, :])
        xq8_sb = consts.tile([128, KT8, BL], FP8)
        nc.sync.dma_start(xq8_sb[:, :, :], aps["xT8"][:, :])

        def dma_w1(e, split=False):
            """fp8 k-pair tile + bf16 tail (two column-half DMAs). With
            split=True only the first half is queued; the returned thunk
            queues the second (lets expert 0's xl DMA slot between them)."""
            w18_sb = w18pool.tile([128, KT8, H1], FP8, tag="w18")
            w18src = aps["w18"][e].rearrange("p (t h) -> p t h", h=H1)
            wf_sb = w1pool.tile([128, KTB, H1], BF16, tag="w1f")
            wsrc = aps["w1b"][e].rearrange("p (t h) -> p t h", h=H1)
            nc.sync.dma_start(wf_sb[:, :, 0 : H1 // 2], wsrc[:, :, 0 : H1 // 2])
            nc.sync.dma_start(w18_sb[:, :, 0 : H1 // 2], w18src[:, :, 0 : H1 // 2])
            def rest():
                nc.sync.dma_start(wf_sb[:, :, H1 // 2 :], wsrc[:, :, H1 // 2 :])
                nc.sync.dma_start(w18_sb[:, :, H1 // 2 :], w18src[:, :, H1 // 2 :])
            if not split:
                rest()
                rest = None
            lhs = lambda m, kt: wf_sb[:, kt, bass.ts(m, 128)]
            return w18_sb, lhs, rest

        def dma_w2(e):
            w28_sb = w2pool.tile([128, KT8, H2], FP8, tag="w28")
            nc.sync.dma_start(w28_sb[:, :, :], aps["w28"][e])
            w2_sb = w2pool.tile([128, KT2 - KT8, H2], BF16, tag="w2")
            w2_src = aps["w2"][e].rearrange("p (t o) -> p t o", o=H2)
            kh = (KT2 - KT8) // 2
            nc.sync.dma_start(w2_sb[:, 0:kh, :], w2_src[:, 0:kh, :])
            nc.sync.dma_start(w2_sb[:, kh:, :], w2_src[:, kh:, :])
            return w28_sb, w2_sb

        gate_sb = consts.tile([128, NB, E], F32)
        acc_sb = consts.tile([128, NB, H2], F32)
        if has_b1:
            b1_sb = consts.tile([128, E * MT1], F32)
            nc.sync.dma_start(b1_sb[:, :], aps["b1t"][:, :])
        if has_b2:
            b2_sb = consts.tile([1, E * H2], BF16)
            nc.sync.dma_start(b2_sb[:, :], aps["b2f"][:, :])
            ones_sb = consts.tile([1, 128], BF16)
            nc.vector.memset(ones_sb[:, :], 1.0)

        w18_sb, w1lhs, w1rest = dma_w1(0, split=True)
        xl_sb = rpool.tile([128, KT, BL], BF16)
        nc.sync.dma_start(xl_sb[:, :, :], aps["xl"][:, :])
        srow10_sb = rpool.tile([10, BL], F32)
        nc.sync.dma_start(srow10_sb[:, :], aps["srow"][:, :])
        io10_sb = rpool.tile([10, 1], F32)
        nc.sync.dma_start(io10_sb[:, :], aps["iota10"][:, :])
        setth_sb = rpool.tile([10, EN], BF16)
        nc.sync.dma_start(setth_sb[:, :], aps["setth"][:, :])
        settl_sb = rpool.tile([10, EN], BF16)
        nc.sync.dma_start(settl_sb[:, :], aps["settl"][:, :])
        w1rest()
        ident_sb = rpool.tile([EN, EN], F32)
        nc.sync.dma_start(ident_sb[:, :], aps["ident"][:, :])
        scolr_sb = rpool.tile([128, NB * EN], F32)
        nc.sync.dma_start(scolr_sb[:, :], aps["scol_rep"][:, :])
        io7_sb = rpool.tile([128, NB * EN], F32)
        nc.sync.dma_start(io7_sb[:, :], aps["iota7"][:, :])
        w28_sb, w2_sb = dma_w2(0)

        # ---- PE warm-up while the DMA engines boot (~13us before the
        # routing inputs land); ramps the HAM clock gate to 8/8. ----------
        warm_sb = rpool.tile([128, 640], BF16)
        nc.vector.memset(warm_sb[:, :], 0.0)
        warm_ps = l1ps.tile([128, 512], F32, tag="ps1", name="warm_ps")
        for _ in range(WARM):
            nc.tensor.matmul(
                warm_ps[:, :], lhsT=warm_sb[:, 0:128], rhs=warm_sb[:, 0:512],
                start=True, stop=True,
            )

        # onehot over embedding rows, bf16 [10, BL]: onehot[r, b] = (scene[b] == r)
        oh16_sb = rpool.tile([10, BL], BF16)
        nc.vector.tensor_scalar(
            out=oh16_sb[:, :], in0=srow10_sb[:, :],
            scalar1=io10_sb[:, 0:1], scalar2=None, op0=ALU.is_equal,
        )

        # ---- routing, reversed layout: psg[j, b] = sum_d sflat[d, j] x[d, b]
        # + sett[scene_b, j], computed as bf16 hi/lo three-term sums.
        # Terms 1-2 need only sfh/sfl+xfull (first in the DMA queue) and run
        # before L1(e0); the xl term and sett run after L1(e0), by which time
        # everything has landed -- the PE never waits on the prologue DMA. --
        psg = l2ps.tile([128, BL], F32, tag="ps2", name="psg")
        for kt in range(KT):
            nc.tensor.matmul(
                psg[0:EN, :], lhsT=sfh_sb[:, kt, :], rhs=xfull_sb[:, kt, :],
                start=(kt == 0), stop=False,
            )
        for kt in range(KT):
            nc.tensor.matmul(
                psg[0:EN, :], lhsT=sfl_sb[:, kt, :], rhs=xfull_sb[:, kt, :],
                start=False, stop=False,
            )

        def routing_tail():
            for kt in range(KT):
                nc.tensor.matmul(
                    psg[0:EN, :], lhsT=sfh_sb[:, kt, :], rhs=xl_sb[:, kt, :],
                    start=False, stop=False,
                )
            nc.tensor.matmul(
                psg[0:EN, :], lhsT=setth_sb[:, :], rhs=oh16_sb[:, :],
                start=False, stop=False,
            )
            nc.tensor.matmul(
                psg[0:EN, :], lhsT=settl_sb[:, :], rhs=oh16_sb[:, :],
                start=False, stop=True,
            )

        def layer1(e, w18, w1lhs):
            """hT[f, b] = relu(sum_d W1[d, f] * xT[d, b] + b1[f]); fp8
            DoubleRow covers k rows [0, FP8_K), bf16 the rest."""
            ht_sb = htpool.tile([128, KT2, BL], BF16, tag="ht")
            ht8_sb = htpool.tile([128, KT8, BL], FP8, tag="ht8")
            for m in range(MT1):
                ps = l1ps.tile([128, BL], F32, tag="ps1")
                for kt in range(KTB):
                    nc.tensor.matmul(
                        ps[:, :],
                        lhsT=w1lhs(m, kt),
                        rhs=xfull_sb[:, KT8 + kt, :],
                        start=(kt == 0), stop=False,
                    )
                nc.tensor.matmul(
                    ps[:, :],
                    lhsT=w18[:, 0:KT8, bass.ts(m, 128)],
                    rhs=xq8_sb[:, 0:KT8, :],
                    start=False, stop=True,
                    perf_mode=DR,
                )
                bias1 = b1_sb[:, e * MT1 + m : e * MT1 + m + 1] if has_b1 else 0.0
                dst = ht8_sb[:, m, :] if m < KT8 else ht_sb[:, m, :]
                nc.scalar.activation(dst, ps[:, :], AF.Relu, bias=bias1)
            return ht_sb, ht8_sb

        def transposes():
            """[49, BL] routing result -> four [128, 49] b-tiles via PE; the
            PSUM->SBUF copy is chunked so transpose t waits only chunk t."""
            gsb = rpool.tile([EN, BL], F32)
            psr = []
            for t in range(NB):
                nc.scalar.copy(gsb[:, bass.ts(t, 128)], psg[0:EN, bass.ts(t, 128)])
            for t in range(NB):
                psr_t = l1ps.tile([128, EN], F32, tag="ps1", name=f"psr{t}")
                nc.tensor.matmul(
                    psr_t[:, :], lhsT=gsb[:, bass.ts(t, 128)], rhs=ident_sb[:, :],
                    is_transpose=True,
                )
                psr.append(psr_t)
            return psr

        def routing_chain(psr):
            """Gate computation, fused over all 4 b-tiles ([128, 4*49])."""
            NE = NB * E  # 28
            gp = rpool.tile([128, NB * EN], F32)
            for t in range(NB):
                nc.scalar.copy(gp[:, bass.ts(t, EN)], psr[t][:, :])
            gp4 = gp.rearrange("p (t e s) -> p (t e) s", s=NS, e=E)
            eex = rpool.tile([128, NB * EN], F32)
            nc.scalar.activation(eex[:, :], gp[:, :], AF.Exp)
            z = rpool.tile([128, NE], F32)
            nc.vector.tensor_reduce(out=z[:, :], in_=eex.rearrange("p (t e s) -> p (t e) s", s=NS, e=E), axis=AX.X, op=ALU.add)
            logz = rpool.tile([128, NE], F32)
            nc.scalar.activation(logz[:, :], z[:, :], AF.Ln)
            sg = rpool.tile([128, NE], F32)
            nc.vector.tensor_reduce(out=sg[:, :], in_=gp4, axis=AX.X, op=ALU.add)
            q = rpool.tile([128, NE], F32)
            nc.vector.scalar_tensor_tensor(
                out=q[:, :], in0=sg[:, :], scalar=1.0 / NS, in1=logz[:, :],
                op0=ALU.mult, op1=ALU.subtract,
            )
            oh = rpool.tile([128, NB * EN], F32)
            nc.vector.tensor_tensor(out=oh[:, :], in0=io7_sb[:, :], in1=scolr_sb[:, :], op=ALU.is_equal)
            gsel = rpool.tile([128, NB * EN], F32)
            nc.vector.tensor_tensor(out=gsel[:, :], in0=gp[:, :], in1=oh[:, :], op=ALU.mult)
            s1s = rpool.tile([128, NE], F32)
            nc.vector.tensor_reduce(out=s1s[:, :], in_=gsel.rearrange("p (t e s) -> p (t e) s", s=NS, e=E), axis=AX.X, op=ALU.add)
            score1 = rpool.tile([128, NE], F32)
            nc.vector.tensor_tensor(out=score1[:, :], in0=s1s[:, :], in1=logz[:, :], op=ALU.subtract)

            lg = rpool.tile([128, NE], F32)
            nc.scalar.activation(lg[:, :], score1[:, :], AF.Exp)     # G at scene, in (0,1)
            el = rpool.tile([128, NE], F32)
            nc.scalar.activation(el[:, :], lg[:, :], AF.Exp)         # softmax numerator
            # per-b-tile scalars ([128,1]) for the reductions' broadcasts
            ssum = rpool.tile([128, NB], F32)
            rs = rpool.tile([128, NB], F32)
            m1 = rpool.tile([128, NB], F32)
            m2 = rpool.tile([128, NB], F32)
            k1 = rpool.tile([128, NE], F32)
            k2 = rpool.tile([128, NE], F32)
            g0 = rpool.tile([128, NE], F32)
            el3 = el.rearrange("p (t e) -> p t e", e=E)
            sc3 = score1.rearrange("p (t e) -> p t e", e=E)
            q3 = q.rearrange("p (t e) -> p t e", e=E)
            nc.vector.tensor_reduce(out=ssum[:, :], in_=el3, axis=AX.X, op=ALU.add)
            nc.vector.reciprocal(rs[:, :], ssum[:, :])
            nc.vector.tensor_reduce(out=m1[:, :], in_=sc3, axis=AX.X, op=ALU.min)
            nc.vector.tensor_reduce(out=m2[:, :], in_=q3, axis=AX.X, op=ALU.min)
            for t in range(NB):
                nc.vector.tensor_scalar(
                    out=k1[:, bass.ts(t, E)], in0=score1[:, bass.ts(t, E)],
                    scalar1=m1[:, t : t + 1], scalar2=None, op0=ALU.is_equal,
                )
                nc.vector.tensor_scalar(
                    out=k2[:, bass.ts(t, E)], in0=q[:, bass.ts(t, E)],
                    scalar1=m2[:, t : t + 1], scalar2=None, op0=ALU.is_equal,
                )
                nc.vector.tensor_scalar(
                    out=g0[:, bass.ts(t, E)], in0=el[:, bass.ts(t, E)],
                    scalar1=rs[:, t : t + 1], scalar2=None, op0=ALU.mult,
                )
            kill = rpool.tile([128, NE], F32)
            nc.vector.tensor_tensor(out=kill[:, :], in0=k1[:, :], in1=k2[:, :], op=ALU.mult)
            sel = rpool.tile([128, NE], F32)
            nc.vector.tensor_scalar(
                out=sel[:, :], in0=kill[:, :], scalar1=-1.0, scalar2=1.0,
                op0=ALU.mult, op1=ALU.add,
            )
            gate_flat = gate_sb.rearrange("p t e -> p (t e)")
            nc.vector.tensor_tensor(out=gate_flat[:, :], in0=g0[:, :], in1=sel[:, :], op=ALU.mult)

        def layer2(e, ht_sb, ht8_sb, w28_sb, w2_sb):
            """out[b, o] = relu(sum_h hT[h, b] * W2[h, o] + b2[o]), gated and
            accumulated into acc_sb; last expert streams the result out."""
            last = e == E - 1
            for mb in range(NB):
                for no in range(NO):
                    ps2 = l2ps.tile([128, 512], F32, tag="ps2")
                    nc.tensor.matmul(
                        ps2[:, :],
                        lhsT=ht8_sb[:, 0:KT8, bass.ts(mb, 128)],
                        rhs=w28_sb[:, 0:KT8, bass.ts(no, 512)],
                        start=True, stop=False,
                        perf_mode=DR,
                    )
                    for kt in range(KT2 - KT8):
                        nc.tensor.matmul(
                            ps2[:, :],
                            lhsT=ht_sb[:, KT8 + kt, bass.ts(mb, 128)],
                            rhs=w2_sb[:, kt, bass.ts(no, 512)],
                            start=False,
                            stop=(kt == KT2 - KT8 - 1 and not has_b2),
                        )
                    if has_b2:
                        nc.tensor.matmul(
                            ps2[:, :],
                            lhsT=ones_sb[:, :],
                            rhs=b2_sb[:, e * H2 + no * 512 : e * H2 + (no + 1) * 512],
                            start=False, stop=True,
                        )
                    gcol = gate_sb[:, mb, e : e + 1]
                    if e == 0:
                        nc.scalar.activation(
                            acc_sb[:, mb, bass.ts(no, 512)], ps2[:, :], AF.Relu, scale=gcol
                        )
                    else:
                        tmp = tmppool.tile([128, 512], F32, tag="tmp")
                        nc.scalar.activation(tmp[:, :], ps2[:, :], AF.Relu, scale=gcol)
                        nc.vector.tensor_tensor(
                            out=acc_sb[:, mb, bass.ts(no, 512)],
                            in0=acc_sb[:, mb, bass.ts(no, 512)],
                            in1=tmp[:, :], op=ALU.add,
                        )
                    # Last batch tile of the last expert: store each 512-col
                    # half as soon as its add lands, shortening the tail.
                    if last and mb == NB - 1:
                        nc.sync.dma_start(
                            aps["out"].rearrange("(t p) o -> p t o", p=128)[
                                :, mb, bass.ts(no, 512)
                            ],
                            acc_sb[:, mb, bass.ts(no, 512)],
                        )
                if last and mb < NB - 1:
                    nc.sync.dma_start(
                        aps["out"].rearrange("(t p) o -> p t o", p=128)[:, mb, :],
                        acc_sb[:, mb, :],
                    )

        # ---- expert 0: L1 (while routing result waits), transpose+gate, L2
        ht_sb, ht8_sb = layer1(0, w18_sb, w1lhs)
        routing_tail()
        psr = transposes()
        routing_chain(psr)
        rpool.release()
        nw18, nw1lhs, _ = dma_w1(1)
        layer2(0, ht_sb, ht8_sb, w28_sb, w2_sb)

        for e in range(1, E):
            w18_sb, w1lhs = nw18, nw1lhs
            w28_sb, w2_sb = dma_w2(e)
            ht_sb, ht8_sb = layer1(e, w18_sb, w1lhs)
            if e < E - 1:
                nw18, nw1lhs, _ = dma_w1(e + 1)
            layer2(e, ht_sb, ht8_sb, w28_sb, w2_sb)


def build(has_b1, has_b2):
    """Build + schedule + compile the Bass program. Returns nc."""
    nc = bacc.Bacc("TRN2", target_bir_lowering=False, debug=False)
    aps = {}
    aps["xh"] = nc.dram_tensor("xh", [128, KT * BL], BF16, kind="ExternalInput").ap()
    aps["xl"] = nc.dram_tensor("xl", [128, KT * BL], BF16, kind="ExternalInput").ap()
    aps["xT8"] = nc.dram_tensor("xT8", [128, KT8 * BL], FP8, kind="ExternalInput").ap()
    aps["w18"] = nc.dram_tensor("w18", [E, 128, KT8 * H1], FP8, kind="ExternalInput").ap()
    aps["w1b"] = nc.dram_tensor("w1b", [E, 128, KTB * H1], BF16, kind="ExternalInput").ap()
    aps["w28"] = nc.dram_tensor("w28", [E, 128, KT8 * H2], FP8, kind="ExternalInput").ap()
    aps["w2"] = nc.dram_tensor("w2", [E, 128, (KT2 - KT8) * H2], BF16, kind="ExternalInput").ap()
    if has_b1:
        aps["b1t"] = nc.dram_tensor("b1t", [128, E * MT1], F32, kind="ExternalInput").ap()
    if has_b2:
        aps["b2f"] = nc.dram_tensor("b2f", [1, E * H2], BF16, kind="ExternalInput").ap()
    aps["sfh"] = nc.dram_tensor("sfh", [128, KT * EN], BF16, kind="ExternalInput").ap()
    aps["sfl"] = nc.dram_tensor("sfl", [128, KT * EN], BF16, kind="ExternalInput").ap()
    aps["setth"] = nc.dram_tensor("setth", [10, EN], BF16, kind="ExternalInput").ap()
    aps["settl"] = nc.dram_tensor("settl", [10, EN], BF16, kind="ExternalInput").ap()
    aps["ident"] = nc.dram_tensor("ident", [EN, EN], F32, kind="ExternalInput").ap()
    aps["scol_rep"] = nc.dram_tensor("scol_rep", [128, NB * EN], F32, kind="ExternalInput").ap()
    aps["srow"] = nc.dram_tensor("srow", [10, BL], F32, kind="ExternalInput").ap()
    aps["iota7"] = nc.dram_tensor("iota7", [128, NB * EN], F32, kind="ExternalInput").ap()
    aps["iota10"] = nc.dram_tensor("iota10", [10, 1], F32, kind="ExternalInput").ap()
    aps["out"] = nc.dram_tensor("out", [BL, H2], F32, kind="ExternalOutput").ap()

    with tile.TileContext(nc) as tc:
        _emit_kernel(tc, aps, has_b1, has_b2)
    nc.compile()
    return nc


def make_in_maps(inputs):
    """Host-side layout prep + batch sharding. Returns (in_maps, has_b1, has_b2)."""
    x = np.ascontiguousarray(np.asarray(inputs["x"], dtype=np.float32))
    scene = np.asarray(inputs["scene"]).astype(np.int64)
    W1 = np.asarray(inputs["W1"], dtype=np.float32)
    b1 = np.asarray(inputs["b1"], dtype=np.float32)
    W2 = np.asarray(inputs["W2"], dtype=np.float32)
    b2 = np.asarray(inputs["b2"], dtype=np.float32)
    S = np.asarray(inputs["S"], dtype=np.float32)
    scene_emb = np.asarray(inputs["scene_emb"], dtype=np.float32)

    has_b1 = bool(np.any(b1))
    has_b2 = bool(np.any(b2))

    def pmaj(a):
        """[T*128, F] -> partition-major [128, T*F]."""
        t = a.shape[0] // 128
        return np.ascontiguousarray(
            a.reshape(t, 128, a.shape[1]).transpose(1, 0, 2).reshape(128, -1)
        )

    def hilo(a):
        h = a.astype(NP_BF16)
        l = (a - h.astype(np.float32)).astype(NP_BF16)
        return np.ascontiguousarray(h), np.ascontiguousarray(l)

    w18 = np.stack([pmaj(W1[e, :FP8_K, :].astype(NP_FP8)) for e in range(E)])
    w1b = np.stack([pmaj(W1[e, FP8_K:, :].astype(NP_BF16)) for e in range(E)])
    w28 = np.stack([pmaj(W2[e, :FP8_K, :].astype(NP_FP8)) for e in range(E)])
    w2b = np.stack([pmaj(W2[e, FP8_K:, :].astype(NP_BF16)) for e in range(E)])
    sflat = np.ascontiguousarray(S[:, :D, :].transpose(1, 2, 0).reshape(D, EN))
    sfh, sfl = hilo(sflat)
    sfh, sfl = pmaj(sfh), pmaj(sfl)
    sett = np.ascontiguousarray(
        np.einsum("rm,sme->res", scene_emb, S[:, D:, :]).reshape(scene_emb.shape[0], EN)
    )
    setth, settl = hilo(sett)
    iota7 = np.ascontiguousarray(np.broadcast_to(
        np.tile(np.arange(EN, dtype=np.float32) % NS, NB).reshape(1, NB * EN),
        (128, NB * EN)))
    iota10 = np.arange(10, dtype=np.float32).reshape(10, 1)
    ident = np.eye(EN, dtype=np.float32)
    shared = {
        "w18": w18, "w1b": w1b, "w28": w28, "w2": w2b, "sfh": sfh, "sfl": sfl,
        "setth": setth, "settl": settl, "ident": ident,
        "iota7": iota7, "iota10": iota10,
    }
    if has_b1:
        shared["b1t"] = np.ascontiguousarray(
            b1.reshape(E, MT1, 128).transpose(2, 0, 1).reshape(128, E * MT1)
        )
    if has_b2:
        shared["b2f"] = np.ascontiguousarray(b2.astype(NP_BF16).reshape(1, E * H2))

    in_maps = []
    for c in range(N_CORES):
        xs = x[c * BL : (c + 1) * BL]
        sc = scene[c * BL : (c + 1) * BL]
        xT = np.ascontiguousarray(xs.T)
        m = dict(shared)
        xhh, xll = hilo(xT)
        m["xh"], m["xl"] = pmaj(xhh), pmaj(xll)
        m["xT8"] = pmaj(np.ascontiguousarray(xT[:FP8_K].astype(NP_FP8)))
        scol = sc.reshape(NB, 128).T.astype(np.float32)          # [128, NB]
        m["scol_rep"] = np.ascontiguousarray(
            np.repeat(scol[:, :, None], EN, axis=2).reshape(128, NB * EN)
        )
        m["srow"] = np.ascontiguousarray(np.broadcast_to(
            sc.astype(np.float32).reshape(1, BL), (10, BL)))
        in_maps.append(m)
    return in_maps, has_b1, has_b2


_NC_CACHE = {}


def get_compiled(has_b1, has_b2):
    key = (has_b1, has_b2)
    if key not in _NC_CACHE:
        _NC_CACHE[key] = build(has_b1, has_b2)
    return _NC_CACHE[key]


def run(inputs, trace=False, **kwargs):
    """Run on hardware; returns (full_output, BassKernelResults)."""
    in_maps, has_b1, has_b2 = make_in_maps(inputs)
    nc = get_compiled(has_b1, has_b2)
    res = run_bass_kernel_spmd(nc, in_maps, core_ids=list(range(N_CORES)), trace=trace, **kwargs)
    parts = [res.results[c]["out"] for c in range(N_CORES)]
    out = np.concatenate(parts, axis=0).astype(np.float32)
    full = np.ascontiguousarray(np.broadcast_to(out[None], (T, B, H2)))
    return full, res


def kernel(**inputs):
    full, _ = run(inputs, trace=False)
    return full
